# revision 31
# baseline (speedup 1.0000x reference)
"""Additive attention kernel for Trainium2, data-parallel over batch on 8 cores.

Reference computation (per batch b):
    q_proj = query @ W2 + b2                      # [U]
    v_proj = values[b] @ W1 + b1                  # [T, U]
    score  = tanh(v_proj + q_proj) @ V + bv       # [T, 1]
    attw   = softmax(score, axis=0)               # [T, 1]
    ctx    = sum(attw * values[b], axis=0)        # [D]

Device strategy (per core, BL=4 batches):
  - values tiles are cast fp32->bf16 during the HBM DMA (SWDGE cast, 4
    row-tiles per DMA to amortize descriptor-generation overhead), then
    transposed on-chip via the DMA xbar (dma_start_transpose) so the
    contraction dim D lands on partitions.
  - v_projT = W1-chunks (stationary) x valuesT (moving) accumulated in PSUM,
    tanh+bias applied by the scalar engine (bias = q_projT column + b1 + b2,
    per-partition), score matvec with V on the tensor engine (M=1 matmuls).
  - softmax skips the max-subtraction: |score| <= sum|V| (tanh bounded), so
    exp cannot overflow; softmax is shift-invariant so results match. The
    exp's free-axis accum_out produces the normalizer without a DVE reduce.
  - context accumulates unnormalized exp weights against natural-layout
    values per half-batch (weight row moved to columns with K=1 matmuls
    against a [[1.0]] constant), and is scaled by 1/l at the end.

bv is ignored: softmax(score + bv) == softmax(score).
"""

import numpy as np

B, T, D, U = 32, 2048, 1024, 1024
NCORES = 8
BL = B // NCORES  # batches per core
KC = D // 128     # contraction chunks
MC = U // 128     # u chunks
NH = 2            # halves per batch (score pass granularity)
TH = T // NH      # rows per half
NN = TH // 512    # 512-col chunks per half
TT = T // 128     # 128-row tiles per batch
GG = 4            # 128-row tiles per cast-DMA group
NG = T // (128 * GG)  # groups per batch

_CACHE = {}


def _build_module():
    import os
    from contextlib import ExitStack

    import concourse.tile as tile
    from concourse import bacc, mybir
    from concourse.bass import ts
    from concourse.bass import _add_dep_helper as add_dep
    from concourse.masks import make_identity

    # bisection knobs (default = current best config)
    kv_deps = os.environ.get("KV_DEPS", "sync")  # sync | nosync | off
    kv_scalar_out = bool(int(os.environ.get("KV_SCALAR_OUT", "1")))
    kv_accum_out = bool(int(os.environ.get("KV_ACCUM_OUT", "1")))

    f32 = mybir.dt.float32
    bf16 = mybir.dt.bfloat16
    Tanh = mybir.ActivationFunctionType.Tanh
    Exp = mybir.ActivationFunctionType.Exp
    X = mybir.AxisListType.X

    nc = bacc.Bacc(
        "TRN2", target_bir_lowering=False, debug=False, num_devices=NCORES
    )
    values = nc.dram_tensor("values", [BL, T, D], f32, kind="ExternalInput").ap()
    query = nc.dram_tensor("query", [BL, D], f32, kind="ExternalInput").ap()
    W1d = nc.dram_tensor("W1", [D, U], f32, kind="ExternalInput").ap()
    W2d = nc.dram_tensor("W2", [D, U], f32, kind="ExternalInput").ap()
    b1d = nc.dram_tensor("b1", [U], f32, kind="ExternalInput").ap()
    b2d = nc.dram_tensor("b2", [U], f32, kind="ExternalInput").ap()
    Vd = nc.dram_tensor("V", [U, 1], f32, kind="ExternalInput").ap()
    ctx_out = nc.dram_tensor("ctx", [BL, D], f32, kind="ExternalOutput").ap()
    attw_out = nc.dram_tensor("attw", [BL, T], f32, kind="ExternalOutput").ap()

    with tile.TileContext(nc) as tc, ExitStack() as ctx:
        consts = ctx.enter_context(tc.tile_pool(name="consts", bufs=1))
        vnat_pool = ctx.enter_context(tc.tile_pool(name="vnat", bufs=2 * NG))
        vt_pool = ctx.enter_context(tc.tile_pool(name="vt", bufs=6))
        tanh_pool = ctx.enter_context(tc.tile_pool(name="tanh", bufs=4))
        sm_pool = ctx.enter_context(tc.tile_pool(name="sm", bufs=2))
        psum_v = ctx.enter_context(tc.tile_pool(name="psv", bufs=3, space="PSUM"))
        psum_s = ctx.enter_context(tc.tile_pool(name="pss", bufs=1, space="PSUM"))
        psum_c = ctx.enter_context(tc.tile_pool(name="psc", bufs=1, space="PSUM"))
        psum_misc = ctx.enter_context(
            tc.tile_pool(name="psm", bufs=1, space="PSUM")
        )

        one_bf = consts.tile([1, 1], bf16)
        nc.vector.memset(one_bf, 1.0)
        eye_bl = consts.tile([BL, BL], f32)
        make_identity(nc, eye_bl)
        eye_mc = consts.tile([MC, MC], f32)
        make_identity(nc, eye_mc)

        # ---- phase 0 inputs (small, issued first) ----
        qn = consts.tile([BL, D], f32)
        nc.sync.dma_start(out=qn, in_=query)
        b1n = consts.tile([MC, 128], f32)
        nc.sync.dma_start(out=b1n, in_=b1d.rearrange("(m p) -> m p", p=128))
        b2n = consts.tile([MC, 128], f32)
        nc.sync.dma_start(out=b2n, in_=b2d.rearrange("(m p) -> m p", p=128))

        # W1 gates all main matmuls: cast-DMA it first
        w1 = consts.tile([128, KC, U], bf16)
        nc.gpsimd.dma_start(out=w1, in_=W1d.rearrange("(k p) u -> p k u", p=128))

        def w1k(k):
            return w1[:, k, :]

        # b12 = (b1 + b2) transposed to partitions via PE: out = b.T @ I
        b12_ps = psum_misc.tile([128, MC], f32, tag="mm")
        nc.tensor.matmul(b12_ps, lhsT=b1n, rhs=eye_mc, start=True, stop=False)
        nc.tensor.matmul(b12_ps, lhsT=b2n, rhs=eye_mc, start=False, stop=True)
        b12 = consts.tile([128, MC], f32)
        nc.vector.tensor_copy(out=b12, in_=b12_ps)

        # queryT via PE: qt[p, k, b] = query[b, k*128+p]
        qt_ps = psum_misc.tile([128, KC, BL], f32, tag="mm")
        for k in range(KC):
            nc.tensor.matmul(
                qt_ps[:, k, :],
                lhsT=qn[:, ts(k, 128)],
                rhs=eye_bl,
                start=True,
                stop=True,
            )
        qt = consts.tile([128, KC, BL], bf16)
        nc.vector.tensor_copy(out=qt, in_=qt_ps)

        # The DMA xbar serializes on every copy<->transpose mode transition
        # (Tile's known-HW-bug workaround), so pin copies and transposes into
        # clean alternating blocks: every copy depends on the previous
        # transpose block so the scheduler can't interleave them.
        last_tblock = []

        def copy_dma(engine, out, in_):
            inst = engine.dma_start(out=out, in_=in_)
            if kv_deps != "off":
                for t in last_tblock:
                    add_dep(
                        inst.ins,
                        t.ins,
                        sync=(kv_deps == "sync"),
                        reason="xbar block order",
                    )
            return inst

        def emit_half_dma(b, half, vnat):
            """casts (GG row-tiles per DMA) + xbar transposes for one half.
            The transposed tiles are split per 512-column quarter so the
            matmuls for n-chunk 0 start after only 4 transposes."""
            nonlocal last_tblock
            vals = values[b].rearrange("(i p) d -> p i d", p=128)
            tblock = []
            vtq = []
            for n in range(NN):
                g = half * NN + n
                vg = vnat_pool.tile([128, GG, D], bf16, tag="vnat")
                copy_dma(nc.gpsimd, vg, vals[:, g * GG : (g + 1) * GG, :])
                vnat.append(vg)
            for n in range(NN):
                vq = vt_pool.tile([128, GG, KC, 128], bf16, tag="vt")
                for tt in range(GG):
                    i = (half * NN + n) * GG + tt
                    tblock.append(
                        nc.sync.dma_start(
                            out=vq[:, tt, :, :],
                            in_=vnat[i // GG][:, i % GG, :],
                            transpose=True,
                        )
                    )
                vtq.append(vq)
            last_tblock = tblock
            return vtq

        def emit_batch_dma(b):
            vnat = []
            vts = [emit_half_dma(b, h, vnat) for h in range(NH)]
            return vnat, vts

        # batch 0 half 0 goes ahead of W2 so the tensor engine can start the
        # main matmuls as early as possible. The first tanhs only need the
        # first q_proj chunks, so W2's first two column-chunks (w2a) load
        # right after batch 0's first tiles; the rest after the transposes.
        w2a = consts.tile([128, KC, 2 * 128], bf16)
        w2b = consts.tile([128, KC, (MC - 2) * 128], bf16)
        w2r = W2d.rearrange("(k p) u -> p k u", p=128)
        vb = consts.tile([128, MC], bf16)

        b0_vnat = []
        b0_vt0 = emit_half_dma(0, 0, b0_vnat)

        copy_dma(nc.gpsimd, w2a, w2r[:, :, : 2 * 128])
        copy_dma(nc.gpsimd, vb, Vd.rearrange("(m p) o -> p (m o)", p=128))
        copy_dma(nc.gpsimd, w2b, w2r[:, :, 2 * 128 :])

        b0_vt1 = emit_half_dma(0, 1, b0_vnat)
        b0_dma = (b0_vnat, [b0_vt0, b0_vt1])

        # q_projT chunks + bias fold: qb[:, m, b] = (query @ W2)^T + b1 + b2
        qb = consts.tile([128, MC, BL], f32)
        for m in range(MC):
            qp_ps = psum_misc.tile([128, BL], f32, tag="mm")
            for k in range(KC):
                w2_lhsT = (
                    w2a[:, k, ts(m, 128)]
                    if m < 2
                    else w2b[:, k, ts(m - 2, 128)]
                )
                nc.tensor.matmul(
                    qp_ps,
                    lhsT=w2_lhsT,
                    rhs=qt[:, k, :],
                    start=(k == 0),
                    stop=(k == KC - 1),
                )
            nc.vector.tensor_scalar_add(
                out=qb[:, m, :], in0=qp_ps, scalar1=b12[:, m : m + 1]
            )

        # ---- main per-batch pipeline ----
        for b in range(BL):
            vnat, vts = b0_dma if b == 0 else emit_batch_dma(b)

            ew_f = sm_pool.tile([1, T], f32, tag="ewf")
            l2 = sm_pool.tile([1, NH], f32, tag="l2")
            ctx_ps = psum_c.tile([1, 2, 512], f32, tag="cx")

            for half in range(NH):
                vt = vts[half]
                sc_ps = psum_s.tile([1, NN, 512], f32, tag="sc")
                for m in range(MC):
                    for n in range(NN):
                        pv = psum_v.tile([128, 512], f32, tag="pv")
                        for k in range(KC):
                            nc.tensor.matmul(
                                pv,
                                lhsT=w1k(k)[:, ts(m, 128)],
                                rhs=vt[n][:, :, k, :],
                                start=(k == 0),
                                stop=(k == KC - 1),
                            )
                        th = tanh_pool.tile([128, 512], bf16, tag="th")
                        nc.scalar.activation(
                            out=th,
                            in_=pv,
                            func=Tanh,
                            bias=qb[:, m, b : b + 1],
                            scale=1.0,
                        )
                        nc.tensor.matmul(
                            sc_ps[0:1, n, :],
                            lhsT=vb[:, m : m + 1],
                            rhs=th,
                            start=(m == 0),
                            stop=(m == MC - 1),
                        )
                # exp (unnormalized) + free-axis sum into l2[half]
                off = half * TH
                if kv_accum_out:
                    nc.scalar.activation(
                        out=ew_f[0:1, off : off + TH].rearrange(
                            "p (a c) -> p a c", a=NN
                        ),
                        in_=sc_ps,
                        func=Exp,
                        accum_out=l2[0:1, half : half + 1],
                    )
                else:
                    for n in range(NN):
                        nc.scalar.activation(
                            out=ew_f[0:1, off + 512 * n : off + 512 * (n + 1)],
                            in_=sc_ps[0:1, n, :],
                            func=Exp,
                        )
                    nc.vector.reduce_sum(
                        out=l2[0:1, half : half + 1],
                        in_=ew_f[0:1, off : off + TH],
                        axis=X,
                    )
                ew_bf = sm_pool.tile([1, TH], bf16, tag="ewbf")
                nc.vector.tensor_copy(out=ew_bf, in_=ew_f[0:1, off : off + TH])

                # weight row -> columns (K=1 matmuls), then context accumulate
                ewT_ps = psum_misc.tile([128, TH // 128], f32, tag="mm")
                for j in range(TH // 128):
                    nc.tensor.matmul(
                        ewT_ps[:, j : j + 1],
                        lhsT=ew_bf[0:1, ts(j, 128)],
                        rhs=one_bf,
                        start=True,
                        stop=True,
                    )
                ew_t = sm_pool.tile([128, TH // 128], bf16, tag="ewt")
                nc.vector.tensor_copy(out=ew_t, in_=ewT_ps)
                for j in range(TH // 128):
                    i = half * (TH // 128) + j
                    for h2 in range(2):
                        nc.tensor.matmul(
                            ctx_ps[0:1, h2, :],
                            lhsT=ew_t[:, j : j + 1],
                            rhs=vnat[i // GG][:, i % GG, ts(h2, 512)],
                            start=(i == 0),
                            stop=(i == TT - 1),
                        )

            # normalize: l = l0 + l1, rl = 1/l
            l_sb = sm_pool.tile([1, 1], f32, tag="l")
            nc.vector.reduce_sum(out=l_sb, in_=l2, axis=X)
            rl = sm_pool.tile([1, 1], f32, tag="rl")
            nc.vector.reciprocal(out=rl, in_=l_sb)
            nc.vector.tensor_scalar_mul(out=ew_f, in0=ew_f, scalar1=rl)
            out_eng = nc.scalar if kv_scalar_out else nc.sync
            copy_dma(out_eng, attw_out[b, :], ew_f)

            ctx_sb = sm_pool.tile([1, D], f32, tag="ctxsb")
            for h2 in range(2):
                nc.vector.tensor_scalar_mul(
                    out=ctx_sb[0:1, ts(h2, 512)],
                    in0=ctx_ps[0:1, h2, :],
                    scalar1=rl,
                )
            copy_dma(out_eng, ctx_out[b, :], ctx_sb)

    nc.compile()
    return nc


def _get_module():
    if "nc" not in _CACHE:
        _CACHE["nc"] = _build_module()
    return _CACHE["nc"]


def kernel(query, values, W1, b1, W2, b2, V, bv):
    import os

    from concourse import bass_utils

    nc = _get_module()

    query = np.ascontiguousarray(np.asarray(query, dtype=np.float32))
    values = np.ascontiguousarray(np.asarray(values, dtype=np.float32))
    W1 = np.ascontiguousarray(np.asarray(W1, dtype=np.float32))
    W2 = np.ascontiguousarray(np.asarray(W2, dtype=np.float32))
    b1 = np.ascontiguousarray(np.asarray(b1, dtype=np.float32))
    b2 = np.ascontiguousarray(np.asarray(b2, dtype=np.float32))
    V = np.ascontiguousarray(np.asarray(V, dtype=np.float32))

    in_maps = []
    for c in range(NCORES):
        sl = slice(c * BL, (c + 1) * BL)
        in_maps.append(
            {
                "values": values[sl],
                "query": query[sl],
                "W1": W1,
                "W2": W2,
                "b1": b1,
                "b2": b2,
                "V": V,
            }
        )

    trace = bool(int(os.environ.get("KERNEL_TRACE", "0")))
    kw = {}
    if os.environ.get("KERNEL_TMPDIR"):
        kw["tmpdir"] = os.environ["KERNEL_TMPDIR"]
    res = bass_utils.run_bass_kernel_spmd(
        nc, in_maps, core_ids=list(range(NCORES)), trace=trace, **kw
    )
    _CACHE["last_res"] = res
    ctx = np.concatenate([res.results[c]["ctx"] for c in range(NCORES)], axis=0)
    attw = np.concatenate(
        [res.results[c]["attw"] for c in range(NCORES)], axis=0
    )
    return ctx.astype(np.float32), attw.reshape(B, T, 1).astype(np.float32)


# revision 41
# speedup vs baseline: 1.0187x; 1.0187x over previous
"""Additive attention kernel for Trainium2, data-parallel over batch on 8 cores.

Reference computation (per batch b):
    q_proj = query @ W2 + b2                      # [U]
    v_proj = values[b] @ W1 + b1                  # [T, U]
    score  = tanh(v_proj + q_proj) @ V + bv       # [T, 1]
    attw   = softmax(score, axis=0)               # [T, 1]
    ctx    = sum(attw * values[b], axis=0)        # [D]

Device strategy (per core, BL=4 batches):
  - values tiles are cast fp32->bf16 during the HBM DMA (SWDGE cast, 4
    row-tiles per DMA to amortize descriptor-generation overhead), then
    transposed on-chip via the DMA xbar (dma_start_transpose) so the
    contraction dim D lands on partitions.
  - v_projT = W1-chunks (stationary) x valuesT (moving) accumulated in PSUM,
    tanh+bias applied by the scalar engine (bias = q_projT column + b1 + b2,
    per-partition), score matvec with V on the tensor engine (M=1 matmuls).
  - softmax skips the max-subtraction: |score| <= sum|V| (tanh bounded), so
    exp cannot overflow; softmax is shift-invariant so results match. The
    exp's free-axis accum_out produces the normalizer without a DVE reduce.
  - context accumulates unnormalized exp weights against natural-layout
    values per half-batch (weight row moved to columns with K=1 matmuls
    against a [[1.0]] constant), and is scaled by 1/l at the end.

bv is ignored: softmax(score + bv) == softmax(score).
"""

import numpy as np

B, T, D, U = 32, 2048, 1024, 1024
NCORES = 8
BL = B // NCORES  # batches per core
KC = D // 128     # contraction chunks
MC = U // 128     # u chunks
NH = 2            # halves per batch (score pass granularity)
TH = T // NH      # rows per half
NN = TH // 512    # 512-col chunks per half
TT = T // 128     # 128-row tiles per batch
GG = 4            # 128-row tiles per cast-DMA group
NG = T // (128 * GG)  # groups per batch

_CACHE = {}


def _build_module():
    import os
    from contextlib import ExitStack

    import concourse.tile as tile
    from concourse import bacc, mybir
    from concourse.bass import ts
    from concourse.bass import _add_dep_helper as add_dep
    from concourse.masks import make_identity

    # bisection knobs (default = current best config)
    kv_deps = os.environ.get("KV_DEPS", "sync")  # sync | nosync | off
    kv_scalar_out = bool(int(os.environ.get("KV_SCALAR_OUT", "0")))
    kv_accum_out = bool(int(os.environ.get("KV_ACCUM_OUT", "1")))

    f32 = mybir.dt.float32
    bf16 = mybir.dt.bfloat16
    Tanh = mybir.ActivationFunctionType.Tanh
    Exp = mybir.ActivationFunctionType.Exp
    X = mybir.AxisListType.X

    nc = bacc.Bacc(
        "TRN2", target_bir_lowering=False, debug=False, num_devices=NCORES
    )
    values = nc.dram_tensor("values", [BL, T, D], f32, kind="ExternalInput").ap()
    query = nc.dram_tensor("query", [BL, D], f32, kind="ExternalInput").ap()
    W1d = nc.dram_tensor("W1", [D, U], f32, kind="ExternalInput").ap()
    W2d = nc.dram_tensor("W2", [D, U], f32, kind="ExternalInput").ap()
    b1d = nc.dram_tensor("b1", [U], f32, kind="ExternalInput").ap()
    b2d = nc.dram_tensor("b2", [U], f32, kind="ExternalInput").ap()
    Vd = nc.dram_tensor("V", [U, 1], f32, kind="ExternalInput").ap()
    ctx_out = nc.dram_tensor("ctx", [BL, D], f32, kind="ExternalOutput").ap()
    attw_out = nc.dram_tensor("attw", [BL, T], f32, kind="ExternalOutput").ap()

    with tile.TileContext(nc) as tc, ExitStack() as ctx:
        consts = ctx.enter_context(tc.tile_pool(name="consts", bufs=1))
        vnat_pool = ctx.enter_context(tc.tile_pool(name="vnat", bufs=2 * NG))
        vt_pool = ctx.enter_context(tc.tile_pool(name="vt", bufs=6))
        tanh_pool = ctx.enter_context(tc.tile_pool(name="tanh", bufs=4))
        sm_pool = ctx.enter_context(tc.tile_pool(name="sm", bufs=2))
        psum_v = ctx.enter_context(tc.tile_pool(name="psv", bufs=3, space="PSUM"))
        psum_s = ctx.enter_context(tc.tile_pool(name="pss", bufs=1, space="PSUM"))
        psum_c = ctx.enter_context(tc.tile_pool(name="psc", bufs=1, space="PSUM"))
        psum_misc = ctx.enter_context(
            tc.tile_pool(name="psm", bufs=1, space="PSUM")
        )

        one_bf = consts.tile([1, 1], bf16)
        nc.vector.memset(one_bf, 1.0)
        eye_bl = consts.tile([BL, BL], f32)
        make_identity(nc, eye_bl)
        eye_mc = consts.tile([MC, MC], f32)
        make_identity(nc, eye_mc)

        # ---- phase 0 inputs (small, issued first) ----
        qn = consts.tile([BL, D], f32)
        nc.sync.dma_start(out=qn, in_=query)
        b1n = consts.tile([MC, 128], f32)
        nc.sync.dma_start(out=b1n, in_=b1d.rearrange("(m p) -> m p", p=128))
        b2n = consts.tile([MC, 128], f32)
        nc.sync.dma_start(out=b2n, in_=b2d.rearrange("(m p) -> m p", p=128))

        # W1 gates all main matmuls: cast-DMA it first
        w1 = consts.tile([128, KC, U], bf16)
        nc.gpsimd.dma_start(out=w1, in_=W1d.rearrange("(k p) u -> p k u", p=128))

        def w1k(k):
            return w1[:, k, :]

        # b12 = (b1 + b2) transposed to partitions via PE: out = b.T @ I
        b12_ps = psum_misc.tile([128, MC], f32, tag="mm")
        nc.tensor.matmul(b12_ps, lhsT=b1n, rhs=eye_mc, start=True, stop=False)
        nc.tensor.matmul(b12_ps, lhsT=b2n, rhs=eye_mc, start=False, stop=True)
        b12 = consts.tile([128, MC], f32)
        nc.vector.tensor_copy(out=b12, in_=b12_ps)

        # queryT via PE: qt[p, k, b] = query[b, k*128+p]
        qt_ps = psum_c.tile([128, KC, BL], f32, tag="cx")
        for k in range(KC):
            nc.tensor.matmul(
                qt_ps[:, k, :],
                lhsT=qn[:, ts(k, 128)],
                rhs=eye_bl,
                start=True,
                stop=True,
            )
        qt = consts.tile([128, KC, BL], bf16)
        nc.vector.tensor_copy(out=qt, in_=qt_ps)

        # The DMA xbar serializes on every copy<->transpose mode transition
        # (Tile's known-HW-bug workaround), so pin copies and transposes into
        # clean alternating blocks: every copy depends on the previous
        # transpose block so the scheduler can't interleave them.
        last_tblock = []

        def copy_dma(engine, out, in_):
            inst = engine.dma_start(out=out, in_=in_)
            if kv_deps != "off":
                for t in last_tblock:
                    add_dep(
                        inst.ins,
                        t.ins,
                        sync=(kv_deps == "sync"),
                        reason="xbar block order",
                    )
            return inst

        def emit_half_dma(b, half, vnat):
            """casts (GG row-tiles per DMA) + xbar transposes for one half.
            The transposed tiles are split per 512-column quarter so the
            matmuls for n-chunk 0 start after only 4 transposes."""
            nonlocal last_tblock
            vals = values[b].rearrange("(i p) d -> p i d", p=128)
            tblock = []
            vtq = []
            for n in range(NN):
                g = half * NN + n
                vg = vnat_pool.tile([128, GG, D], bf16, tag="vnat")
                copy_dma(nc.gpsimd, vg, vals[:, g * GG : (g + 1) * GG, :])
                vnat.append(vg)
            for n in range(NN):
                vq = vt_pool.tile([128, GG, KC, 128], bf16, tag="vt")
                for tt in range(GG):
                    i = (half * NN + n) * GG + tt
                    tblock.append(
                        nc.sync.dma_start(
                            out=vq[:, tt, :, :],
                            in_=vnat[i // GG][:, i % GG, :],
                            transpose=True,
                        )
                    )
                vtq.append(vq)
            last_tblock = tblock
            return vtq

        def emit_batch_dma(b):
            vnat = []
            vts = [emit_half_dma(b, h, vnat) for h in range(NH)]
            return vnat, vts

        # batch 0 half 0 goes ahead of W2 so the tensor engine can start the
        # main matmuls as early as possible. The first tanhs only need the
        # first q_proj chunks, so W2's first two column-chunks (w2a) load
        # right after batch 0's first tiles; the rest after the transposes.
        w2a = consts.tile([128, KC, 2 * 128], bf16)
        w2b = consts.tile([128, KC, (MC - 2) * 128], bf16)
        w2r = W2d.rearrange("(k p) u -> p k u", p=128)
        vb = consts.tile([128, MC], bf16)

        b0_vnat = []
        b0_vt0 = emit_half_dma(0, 0, b0_vnat)

        copy_dma(nc.gpsimd, w2a, w2r[:, :, : 2 * 128])
        copy_dma(nc.gpsimd, vb, Vd.rearrange("(m p) o -> p (m o)", p=128))
        copy_dma(nc.gpsimd, w2b, w2r[:, :, 2 * 128 :])

        b0_vt1 = emit_half_dma(0, 1, b0_vnat)
        b0_dma = (b0_vnat, [b0_vt0, b0_vt1])

        # q_projT chunks + bias fold: qb[:, m, b] = (query @ W2)^T + b1 + b2
        qb = consts.tile([128, MC, BL], f32)
        for m in range(MC):
            qp_ps = psum_s.tile([128, BL], f32, tag="sc")
            for k in range(KC):
                w2_lhsT = (
                    w2a[:, k, ts(m, 128)]
                    if m < 2
                    else w2b[:, k, ts(m - 2, 128)]
                )
                nc.tensor.matmul(
                    qp_ps,
                    lhsT=w2_lhsT,
                    rhs=qt[:, k, :],
                    start=(k == 0),
                    stop=(k == KC - 1),
                )
            nc.vector.tensor_scalar_add(
                out=qb[:, m, :], in0=qp_ps, scalar1=b12[:, m : m + 1]
            )

        # ---- main per-batch pipeline ----
        # output DMAs are deferred until after the NEXT batch's transposes are
        # queued: emitting them earlier puts their (long) input-wait into the
        # xbar copy/transpose chain and stalls the next batch's transposes.
        deferred_outs = []

        def flush_outs():
            while deferred_outs:
                dst, src = deferred_outs.pop(0)
                copy_dma(out_eng, dst, src)

        out_eng = nc.scalar if kv_scalar_out else nc.sync

        for b in range(BL):
            vnat, vts = b0_dma if b == 0 else emit_batch_dma(b)
            flush_outs()

            ew_f = sm_pool.tile([1, T], f32, tag="ewf")
            l2 = sm_pool.tile([1, NH], f32, tag="l2")
            ctx_ps = psum_c.tile([1, 2, 512], f32, tag="cx")

            for half in range(NH):
                vt = vts[half]
                sc_ps = psum_s.tile([1, NN, 512], f32, tag="sc")
                for m in range(MC):
                    for n in range(NN):
                        pv = psum_v.tile([128, 512], f32, tag="pv")
                        for k in range(KC):
                            nc.tensor.matmul(
                                pv,
                                lhsT=w1k(k)[:, ts(m, 128)],
                                rhs=vt[n][:, :, k, :],
                                start=(k == 0),
                                stop=(k == KC - 1),
                            )
                        th = tanh_pool.tile([128, 512], bf16, tag="th")
                        nc.scalar.activation(
                            out=th,
                            in_=pv,
                            func=Tanh,
                            bias=qb[:, m, b : b + 1],
                            scale=1.0,
                        )
                        nc.tensor.matmul(
                            sc_ps[0:1, n, :],
                            lhsT=vb[:, m : m + 1],
                            rhs=th,
                            start=(m == 0),
                            stop=(m == MC - 1),
                        )
                # exp (unnormalized) + free-axis sum into l2[half]
                off = half * TH
                if kv_accum_out:
                    nc.scalar.activation(
                        out=ew_f[0:1, off : off + TH].rearrange(
                            "p (a c) -> p a c", a=NN
                        ),
                        in_=sc_ps,
                        func=Exp,
                        accum_out=l2[0:1, half : half + 1],
                    )
                else:
                    for n in range(NN):
                        nc.scalar.activation(
                            out=ew_f[0:1, off + 512 * n : off + 512 * (n + 1)],
                            in_=sc_ps[0:1, n, :],
                            func=Exp,
                        )
                    nc.vector.reduce_sum(
                        out=l2[0:1, half : half + 1],
                        in_=ew_f[0:1, off : off + TH],
                        axis=X,
                    )
                ew_bf = sm_pool.tile([1, TH], bf16, tag="ewbf")
                nc.vector.tensor_copy(out=ew_bf, in_=ew_f[0:1, off : off + TH])

                # weight row -> columns (K=1 matmuls), then context accumulate
                ewT_ps = psum_misc.tile([128, TH // 128], f32, tag="mm")
                for j in range(TH // 128):
                    nc.tensor.matmul(
                        ewT_ps[:, j : j + 1],
                        lhsT=ew_bf[0:1, ts(j, 128)],
                        rhs=one_bf,
                        start=True,
                        stop=True,
                    )
                ew_t = sm_pool.tile([128, TH // 128], bf16, tag="ewt")
                nc.vector.tensor_copy(out=ew_t, in_=ewT_ps)
                for j in range(TH // 128):
                    i = half * (TH // 128) + j
                    for h2 in range(2):
                        nc.tensor.matmul(
                            ctx_ps[0:1, h2, :],
                            lhsT=ew_t[:, j : j + 1],
                            rhs=vnat[i // GG][:, i % GG, ts(h2, 512)],
                            start=(i == 0),
                            stop=(i == TT - 1),
                        )

            # normalize: l = l0 + l1, rl = 1/l
            l_sb = sm_pool.tile([1, 1], f32, tag="l")
            nc.vector.reduce_sum(out=l_sb, in_=l2, axis=X)
            rl = sm_pool.tile([1, 1], f32, tag="rl")
            nc.vector.reciprocal(out=rl, in_=l_sb)
            nc.vector.tensor_scalar_mul(out=ew_f, in0=ew_f, scalar1=rl)
            deferred_outs.append((attw_out[b, :], ew_f))

            ctx_sb = sm_pool.tile([1, D], f32, tag="ctxsb")
            for h2 in range(2):
                nc.vector.tensor_scalar_mul(
                    out=ctx_sb[0:1, ts(h2, 512)],
                    in0=ctx_ps[0:1, h2, :],
                    scalar1=rl,
                )
            deferred_outs.append((ctx_out[b, :], ctx_sb))
        flush_outs()

    nc.compile()
    return nc


def _get_module():
    if "nc" not in _CACHE:
        _CACHE["nc"] = _build_module()
    return _CACHE["nc"]


def kernel(query, values, W1, b1, W2, b2, V, bv):
    import os

    from concourse import bass_utils

    nc = _get_module()

    query = np.ascontiguousarray(np.asarray(query, dtype=np.float32))
    values = np.ascontiguousarray(np.asarray(values, dtype=np.float32))
    W1 = np.ascontiguousarray(np.asarray(W1, dtype=np.float32))
    W2 = np.ascontiguousarray(np.asarray(W2, dtype=np.float32))
    b1 = np.ascontiguousarray(np.asarray(b1, dtype=np.float32))
    b2 = np.ascontiguousarray(np.asarray(b2, dtype=np.float32))
    V = np.ascontiguousarray(np.asarray(V, dtype=np.float32))

    in_maps = []
    for c in range(NCORES):
        sl = slice(c * BL, (c + 1) * BL)
        in_maps.append(
            {
                "values": values[sl],
                "query": query[sl],
                "W1": W1,
                "W2": W2,
                "b1": b1,
                "b2": b2,
                "V": V,
            }
        )

    trace = bool(int(os.environ.get("KERNEL_TRACE", "0")))
    kw = {}
    if os.environ.get("KERNEL_TMPDIR"):
        kw["tmpdir"] = os.environ["KERNEL_TMPDIR"]
    res = bass_utils.run_bass_kernel_spmd(
        nc, in_maps, core_ids=list(range(NCORES)), trace=trace, **kw
    )
    _CACHE["last_res"] = res
    ctx = np.concatenate([res.results[c]["ctx"] for c in range(NCORES)], axis=0)
    attw = np.concatenate(
        [res.results[c]["attw"] for c in range(NCORES)], axis=0
    )
    return ctx.astype(np.float32), attw.reshape(B, T, 1).astype(np.float32)


# revision 56
# speedup vs baseline: 1.3783x; 1.3530x over previous
"""Additive attention kernel for Trainium2, data-parallel over batch on 8 cores.

Reference computation (per batch b):
    q_proj = query @ W2 + b2                      # [U]
    v_proj = values[b] @ W1 + b1                  # [T, U]
    score  = tanh(v_proj + q_proj) @ V + bv       # [T, 1]
    attw   = softmax(score, axis=0)               # [T, 1]
    ctx    = sum(attw * values[b], axis=0)        # [D]

Device strategy (per core, BL=4 batches):
  - values tiles are cast fp32->bf16 during the HBM DMA (SWDGE cast, 4
    row-tiles per DMA to amortize descriptor-generation overhead), then
    transposed on-chip via the DMA xbar (dma_start_transpose) so the
    contraction dim D lands on partitions.
  - v_projT = W1-chunks (stationary) x valuesT (moving) accumulated in PSUM,
    tanh+bias applied by the scalar engine (bias = q_projT column + b1 + b2,
    per-partition), score matvec with V on the tensor engine (M=1 matmuls).
  - softmax skips the max-subtraction: |score| <= sum|V| (tanh bounded), so
    exp cannot overflow; softmax is shift-invariant so results match. The
    exp's free-axis accum_out produces the normalizer without a DVE reduce.
  - context accumulates unnormalized exp weights against natural-layout
    values per half-batch (weight row moved to columns with K=1 matmuls
    against a [[1.0]] constant), and is scaled by 1/l at the end.

bv is ignored: softmax(score + bv) == softmax(score).
"""

import numpy as np

B, T, D, U = 32, 2048, 1024, 1024
NCORES = 8
BL = B // NCORES  # batches per core
KC = D // 128     # contraction chunks
MC = U // 128     # u chunks
NH = 2            # halves per batch (score pass granularity)
TH = T // NH      # rows per half
NN = TH // 512    # 512-col chunks per half
TT = T // 128     # 128-row tiles per batch
GG = 4            # 128-row tiles per cast-DMA group
NG = T // (128 * GG)  # groups per batch

_CACHE = {}


def _build_module():
    import os
    from contextlib import ExitStack

    import concourse.tile as tile
    from concourse import bacc, mybir
    from concourse.bass import ts
    from concourse.bass import _add_dep_helper as add_dep
    from concourse.masks import make_identity

    # bisection knobs (default = current best config)
    kv_deps = os.environ.get("KV_DEPS", "sync")  # sync | nosync | off
    kv_scalar_out = bool(int(os.environ.get("KV_SCALAR_OUT", "0")))
    kv_accum_out = bool(int(os.environ.get("KV_ACCUM_OUT", "1")))

    f32 = mybir.dt.float32
    bf16 = mybir.dt.bfloat16
    Tanh = mybir.ActivationFunctionType.Tanh
    Exp = mybir.ActivationFunctionType.Exp
    X = mybir.AxisListType.X

    nc = bacc.Bacc(
        "TRN2", target_bir_lowering=False, debug=False, num_devices=NCORES
    )
    values = nc.dram_tensor("values", [BL, T, D], f32, kind="ExternalInput").ap()
    query = nc.dram_tensor("query", [BL, D], f32, kind="ExternalInput").ap()
    W1d = nc.dram_tensor("W1", [D, U], f32, kind="ExternalInput").ap()
    W2d = nc.dram_tensor("W2", [D, U], f32, kind="ExternalInput").ap()
    b1d = nc.dram_tensor("b1", [U], f32, kind="ExternalInput").ap()
    b2d = nc.dram_tensor("b2", [U], f32, kind="ExternalInput").ap()
    Vd = nc.dram_tensor("V", [U, 1], f32, kind="ExternalInput").ap()
    ctx_out = nc.dram_tensor("ctx", [BL, D], f32, kind="ExternalOutput").ap()
    attw_out = nc.dram_tensor("attw", [BL, T], f32, kind="ExternalOutput").ap()

    with tile.TileContext(nc) as tc, ExitStack() as ctx:
        consts = ctx.enter_context(tc.tile_pool(name="consts", bufs=1))
        vnat_pool = ctx.enter_context(tc.tile_pool(name="vnat", bufs=2 * NG))
        vt_pool = ctx.enter_context(tc.tile_pool(name="vt", bufs=6))
        tanh_pool = ctx.enter_context(tc.tile_pool(name="tanh", bufs=6))
        sm_pool = ctx.enter_context(tc.tile_pool(name="sm", bufs=2))
        psum_v = ctx.enter_context(tc.tile_pool(name="psv", bufs=3, space="PSUM"))
        psum_s = ctx.enter_context(tc.tile_pool(name="pss", bufs=1, space="PSUM"))
        psum_c = ctx.enter_context(tc.tile_pool(name="psc", bufs=1, space="PSUM"))
        psum_misc = ctx.enter_context(
            tc.tile_pool(name="psm", bufs=1, space="PSUM")
        )

        one_bf = consts.tile([1, 1], bf16)
        nc.vector.memset(one_bf, 1.0)
        # touch the Tanh/Exp activation-table set early so its ~2.7us load
        # happens during the initial DMA fill, not before the first real tanh
        warm = consts.tile([1, 1], f32)
        nc.vector.memset(warm, 0.0)
        nc.scalar.activation(out=warm, in_=warm, func=Tanh)
        eye_bl = consts.tile([BL, BL], f32)
        make_identity(nc, eye_bl)
        eye_mc = consts.tile([MC, MC], f32)
        make_identity(nc, eye_mc)

        # ---- phase 0 inputs (small, issued first) ----
        qn = consts.tile([BL, D], f32)
        nc.sync.dma_start(out=qn, in_=query)
        b1n = consts.tile([MC, 128], f32)
        nc.sync.dma_start(out=b1n, in_=b1d.rearrange("(m p) -> m p", p=128))
        b2n = consts.tile([MC, 128], f32)
        nc.sync.dma_start(out=b2n, in_=b2d.rearrange("(m p) -> m p", p=128))

        # the first two W2 column-chunks (tanh bias for m=0,1) are small:
        # load them before W1 so q_proj chunks are ready when tanh starts
        w2r = W2d.rearrange("(k p) u -> p k u", p=128)
        # W1 gates all main matmuls: cast-DMA it first
        w1 = consts.tile([128, KC, U], bf16)
        nc.gpsimd.dma_start(out=w1, in_=W1d.rearrange("(k p) u -> p k u", p=128))

        def w1k(k):
            return w1[:, k, :]

        # b12 = (b1 + b2) transposed to partitions via PE: out = b.T @ I
        b12_ps = psum_misc.tile([128, MC], f32, tag="mm")
        nc.tensor.matmul(b12_ps, lhsT=b1n, rhs=eye_mc, start=True, stop=False)
        nc.tensor.matmul(b12_ps, lhsT=b2n, rhs=eye_mc, start=False, stop=True)
        b12 = consts.tile([128, MC], f32)
        nc.vector.tensor_copy(out=b12, in_=b12_ps)

        # queryT via PE: qt[p, k, b] = query[b, k*128+p]
        qt_ps = psum_c.tile([128, KC, BL], f32, tag="cx")
        for k in range(KC):
            nc.tensor.matmul(
                qt_ps[:, k, :],
                lhsT=qn[:, ts(k, 128)],
                rhs=eye_bl,
                start=True,
                stop=True,
            )
        qt = consts.tile([128, KC, BL], bf16)
        nc.vector.tensor_copy(out=qt, in_=qt_ps)

        # The DMA xbar serializes on every copy<->transpose mode transition
        # (Tile's known-HW-bug workaround), so pin copies and transposes into
        # clean alternating blocks: every copy depends on the previous
        # transpose block so the scheduler can't interleave them.
        last_tblock = []

        def copy_dma(engine, out, in_):
            inst = engine.dma_start(out=out, in_=in_)
            if kv_deps != "off":
                for t in last_tblock:
                    add_dep(
                        inst.ins,
                        t.ins,
                        sync=(kv_deps == "sync"),
                        reason="xbar block order",
                    )
            return inst

        def emit_half_dma(b, half, vnat):
            """casts (GG row-tiles per DMA) + xbar transposes for one half.
            The transposed tiles are split per 512-column quarter so the
            matmuls for n-chunk 0 start after only 4 transposes."""
            nonlocal last_tblock
            vals = values[b].rearrange("(i p) d -> p i d", p=128)
            tblock = []
            vtq = []
            for n in range(NN):
                g = half * NN + n
                vg = vnat_pool.tile([128, GG, D], bf16, tag="vnat")
                copy_dma(nc.gpsimd, vg, vals[:, g * GG : (g + 1) * GG, :])
                vnat.append(vg)
            for n in range(NN):
                vq = vt_pool.tile([128, GG, KC, 128], bf16, tag="vt")
                for tt in range(GG):
                    i = (half * NN + n) * GG + tt
                    tblock.append(
                        nc.sync.dma_start(
                            out=vq[:, tt, :, :],
                            in_=vnat[i // GG][:, i % GG, :],
                            transpose=True,
                        )
                    )
                vtq.append(vq)
            last_tblock = tblock
            return vtq

        def emit_batch_dma(b):
            vnat = []
            vts = [emit_half_dma(b, h, vnat) for h in range(NH)]
            return vnat, vts

        # batch 0 half 0 goes ahead of W2 so the tensor engine can start the
        # main matmuls as early as possible. The first tanhs only need the
        # first q_proj chunks, so W2's first two column-chunks (w2a) load
        # right after batch 0's first tiles; the rest after the transposes.
        w2a = consts.tile([128, KC, 2 * 128], bf16)
        w2b = consts.tile([128, KC, (MC - 2) * 128], bf16)
        vb = consts.tile([128, MC], bf16)

        b0_vnat = []
        b0_vt0 = emit_half_dma(0, 0, b0_vnat)

        copy_dma(nc.gpsimd, w2a, w2r[:, :, : 2 * 128])
        copy_dma(nc.gpsimd, vb, Vd.rearrange("(m p) o -> p (m o)", p=128))
        copy_dma(nc.gpsimd, w2b, w2r[:, :, 2 * 128 :])

        b0_vt1 = emit_half_dma(0, 1, b0_vnat)
        b0_dma = (b0_vnat, [b0_vt0, b0_vt1])

        # q_projT chunks + bias fold: qb[:, m, b] = (query @ W2)^T + b1 + b2
        qb = consts.tile([128, MC, BL], f32)
        for m in range(MC):
            qp_ps = psum_s.tile([128, BL], f32, tag="sc")
            for k in range(KC):
                w2_lhsT = (
                    w2a[:, k, ts(m, 128)]
                    if m < 2
                    else w2b[:, k, ts(m - 2, 128)]
                )
                nc.tensor.matmul(
                    qp_ps,
                    lhsT=w2_lhsT,
                    rhs=qt[:, k, :],
                    start=(k == 0),
                    stop=(k == KC - 1),
                )
            nc.vector.tensor_scalar_add(
                out=qb[:, m, :], in0=qp_ps, scalar1=b12[:, m : m + 1]
            )

        # ---- main per-batch pipeline ----
        # output DMAs are deferred until after the NEXT batch's transposes are
        # queued: emitting them earlier puts their (long) input-wait into the
        # xbar copy/transpose chain and stalls the next batch's transposes.
        deferred_outs = []

        def flush_outs():
            while deferred_outs:
                dst, src = deferred_outs.pop(0)
                copy_dma(out_eng, dst, src)

        out_eng = nc.scalar if kv_scalar_out else nc.sync

        for b in range(BL):
            vnat, vts = b0_dma if b == 0 else emit_batch_dma(b)
            flush_outs()

            ew_f = sm_pool.tile([1, T], f32, tag="ewf")
            l2 = sm_pool.tile([1, NH], f32, tag="l2")
            ctx_ps = psum_c.tile([1, 2, 512], f32, tag="cx")

            for half in range(NH):
                vt = vts[half]
                sc_ps = psum_s.tile([1, NN, 512], f32, tag="sc")
                for m in range(MC):
                    for n in range(NN):
                        pv = psum_v.tile([128, 512], f32, tag="pv")
                        for k in range(KC):
                            nc.tensor.matmul(
                                pv,
                                lhsT=w1k(k)[:, ts(m, 128)],
                                rhs=vt[n][:, :, k, :],
                                start=(k == 0),
                                stop=(k == KC - 1),
                            )
                        th = tanh_pool.tile([128, 512], bf16, tag="th")
                        nc.scalar.activation(
                            out=th,
                            in_=pv,
                            func=Tanh,
                            bias=qb[:, m, b : b + 1],
                            scale=1.0,
                        )
                        nc.tensor.matmul(
                            sc_ps[0:1, n, :],
                            lhsT=vb[:, m : m + 1],
                            rhs=th,
                            start=(m == 0),
                            stop=(m == MC - 1),
                        )
                # exp (unnormalized) + free-axis sum into l2[half]
                off = half * TH
                if kv_accum_out:
                    nc.scalar.activation(
                        out=ew_f[0:1, off : off + TH].rearrange(
                            "p (a c) -> p a c", a=NN
                        ),
                        in_=sc_ps,
                        func=Exp,
                        accum_out=l2[0:1, half : half + 1],
                    )
                else:
                    for n in range(NN):
                        nc.scalar.activation(
                            out=ew_f[0:1, off + 512 * n : off + 512 * (n + 1)],
                            in_=sc_ps[0:1, n, :],
                            func=Exp,
                        )
                    nc.vector.reduce_sum(
                        out=l2[0:1, half : half + 1],
                        in_=ew_f[0:1, off : off + TH],
                        axis=X,
                    )
                ew_bf = sm_pool.tile([1, TH], bf16, tag="ewbf")
                nc.vector.tensor_copy(out=ew_bf, in_=ew_f[0:1, off : off + TH])

                # weight row -> columns (K=1 matmuls), then context accumulate
                ewT_ps = psum_misc.tile([128, TH // 128], f32, tag="mm")
                for j in range(TH // 128):
                    nc.tensor.matmul(
                        ewT_ps[:, j : j + 1],
                        lhsT=ew_bf[0:1, ts(j, 128)],
                        rhs=one_bf,
                        start=True,
                        stop=True,
                    )
                ew_t = sm_pool.tile([128, TH // 128], bf16, tag="ewt")
                nc.vector.tensor_copy(out=ew_t, in_=ewT_ps)
                for j in range(TH // 128):
                    i = half * (TH // 128) + j
                    for h2 in range(2):
                        nc.tensor.matmul(
                            ctx_ps[0:1, h2, :],
                            lhsT=ew_t[:, j : j + 1],
                            rhs=vnat[i // GG][:, i % GG, ts(h2, 512)],
                            start=(i == 0),
                            stop=(i == TT - 1),
                        )

            # normalize: l = l0 + l1, rl = 1/l
            l_sb = sm_pool.tile([1, 1], f32, tag="l")
            nc.vector.reduce_sum(out=l_sb, in_=l2, axis=X)
            rl = sm_pool.tile([1, 1], f32, tag="rl")
            nc.vector.reciprocal(out=rl, in_=l_sb)
            nc.vector.tensor_scalar_mul(out=ew_f, in0=ew_f, scalar1=rl)
            deferred_outs.append((attw_out[b, :], ew_f))

            ctx_sb = sm_pool.tile([1, D], f32, tag="ctxsb")
            for h2 in range(2):
                nc.vector.tensor_scalar_mul(
                    out=ctx_sb[0:1, ts(h2, 512)],
                    in0=ctx_ps[0:1, h2, :],
                    scalar1=rl,
                )
            deferred_outs.append((ctx_out[b, :], ctx_sb))
        flush_outs()

    nc.compile()
    return nc


def _get_module():
    if "nc" not in _CACHE:
        _CACHE["nc"] = _build_module()
    return _CACHE["nc"]


def kernel(query, values, W1, b1, W2, b2, V, bv):
    import os

    from concourse import bass_utils

    nc = _get_module()

    query = np.ascontiguousarray(np.asarray(query, dtype=np.float32))
    values = np.ascontiguousarray(np.asarray(values, dtype=np.float32))
    W1 = np.ascontiguousarray(np.asarray(W1, dtype=np.float32))
    W2 = np.ascontiguousarray(np.asarray(W2, dtype=np.float32))
    b1 = np.ascontiguousarray(np.asarray(b1, dtype=np.float32))
    b2 = np.ascontiguousarray(np.asarray(b2, dtype=np.float32))
    V = np.ascontiguousarray(np.asarray(V, dtype=np.float32))

    in_maps = []
    for c in range(NCORES):
        sl = slice(c * BL, (c + 1) * BL)
        in_maps.append(
            {
                "values": values[sl],
                "query": query[sl],
                "W1": W1,
                "W2": W2,
                "b1": b1,
                "b2": b2,
                "V": V,
            }
        )

    trace = bool(int(os.environ.get("KERNEL_TRACE", "0")))
    kw = {}
    if os.environ.get("KERNEL_TMPDIR"):
        kw["tmpdir"] = os.environ["KERNEL_TMPDIR"]
    res = bass_utils.run_bass_kernel_spmd(
        nc, in_maps, core_ids=list(range(NCORES)), trace=trace, **kw
    )
    _CACHE["last_res"] = res
    ctx = np.concatenate([res.results[c]["ctx"] for c in range(NCORES)], axis=0)
    attw = np.concatenate(
        [res.results[c]["attw"] for c in range(NCORES)], axis=0
    )
    return ctx.astype(np.float32), attw.reshape(B, T, 1).astype(np.float32)
